# revision 13
# baseline (speedup 1.0000x reference)
"""Trainium2 Bass kernel for nn_BezierButtress (Bernstein-basis permutation chains).

Math (per permutation chain p, over depth d = 0..31):
    S_mean <- (S_mean @ Wm_d) * B(x_{perm[p,d]})        (K=17 wide state)
    S_var  <- (S_var  @ Wv_d) * B(x_{perm[p,d]})^2
    outputs: f_mean[n] = sum_{p,k} S_mean, f_var[n] = sum_{p,k} S_var / post_prec[p]

Device strategy (data-parallel over N across 8 cores, 3072 rows each):
  * state layout: (7 chains x 17 k -> 128 partitions incl. pad, n free),
    block-diagonal 128x128 fp32r chain matmuls (3 groups cover 20 chains).
  * per-step Bernstein multipliers built in log space: one PE matmul contracts
    a baked selection/coefficient matrix A_{d,g} (128 x 128) against a resident
    log-table UV (U_hi/V_hi/U_lo/V_lo, 128 x n) giving
    logM = k*log(x_c) + (16-k)*log(1-x_c) exactly (hi/lo splitting cancels the
    PE fp22 truncation); then ACT computes exp(logM + log binom); the squared
    multiplier comes from exp(scale=2) on ACT or an SBUF square on GPSIMD.
  * meanw0 / exp(varw0)*sc2 / sc2 column scale / 1/post_prec are all folded
    host-side into the baked block-diagonal weights & reduction vectors
    (weights pre-rounded to e10m11 so the PE fp32r truncation is a no-op).
  * emission is software-pipelined one tile ahead (gather of tile t+1 before
    compute of tile t) over a double-buffered 4-bank PSUM tile whose banks are
    reused logM -> chain outputs within each tile's lifetime.
"""

import os
import numpy as np
import ml_dtypes
from math import comb

import concourse.bass as bass
import concourse.mybir as mybir
import concourse.tile as tile
from concourse import bacc
from concourse import bass_utils

ORDER = 16
K = 17
D = 32
P = 20
N = 24576
NCORES = 8
NLOC = N // NCORES        # 3072
CPG = 7                   # chain slots per group
G = 3                     # groups (7, 7, 6 + 1 pad)
R = CPG * K               # 119 active partitions
RP = 128                  # padded partition count
CHUNK = 1024
HALF = 512
F32 = mybir.dt.float32
F32R = mybir.dt.float32r
BF16 = mybir.dt.bfloat16
EXP = mybir.ActivationFunctionType.Exp
MULT = mybir.AluOpType.mult


def _flags():
    # NOTE: walrus rejects mixed 32-bit/16-bit matmul inputs (NCC_IBIR034),
    # so bf16 operands require BOTH sides bf16.  a16=2 runs the whole gather
    # matmul (A and the UV log-table) in bf16 -- A entries are small exact
    # integers and UV is hi/lo split, so the effective log-table mantissa is
    # ~16 bits; bf16 streams ~2x faster through the PE than fp32r.
    a16 = int(os.environ.get("BB_A16", "2"))         # 0=f32r, 2=bf16 gather
    w16 = bool(int(os.environ.get("BB_W16", "0")))   # bf16 hi/lo weights (invalid)
    gp3 = int(os.environ.get("BB_GP3", "2"))         # GP square 2-of-3 tiles
    mulmod = int(os.environ.get("BB_MULMOD", "6"))   # 1-in-mulmod muls via ACT+GP
    return a16, w16, gp3, mulmod


def _fp22_round(x64):
    """Round float64 to the nearest fp22 (e10m11) value, returned as float32.
    The PE's fp32r path *truncates* inputs to fp22; feeding it pre-rounded
    values makes that truncation a no-op and kills the systematic bias."""
    x32 = x64.astype(np.float32)
    u = x32.view(np.uint32).astype(np.uint64)
    u = ((u + 0x800) & 0xFFFFF000).astype(np.uint32)   # round-half-up on m11
    return u.view(np.float32)


def _fp22_split(x64):
    """Split float64 -> (hi, lo) float32 with hi exactly representable in
    fp22 (e10m11), so PE fp32r matmuls consume hi/lo exactly."""
    x32 = x64.astype(np.float32)
    hi = (x32.view(np.uint32) & np.uint32(0xFFFFF000)).view(np.float32)
    lo = (x64 - hi.astype(np.float64)).astype(np.float32)
    return hi, lo


def _bf16_split(x64):
    hi = x64.astype(ml_dtypes.bfloat16)
    lo = (x64 - hi.astype(np.float64)).astype(ml_dtypes.bfloat16)
    return hi, lo


def _host_tensors(Xnew, meanw0, meanw, varw0, varw, prior_sc, post_prec, perm):
    a16, w16, _, _ = _flags()
    Xnew = np.asarray(Xnew, np.float32)
    meanw0 = np.asarray(meanw0, np.float64)   # (P, 1, K)
    meanw = np.asarray(meanw, np.float64)     # (D-1, P, K, K)
    varw0 = np.asarray(varw0, np.float64)     # (P, 1, K)
    varw = np.asarray(varw, np.float64)       # (D-1, P, K, K)
    prior_sc = np.asarray(prior_sc, np.float64)  # (K, 1)
    post_prec = np.asarray(post_prec, np.float64)  # (P,)
    perm = np.asarray(perm)                   # (P, D) int

    # --- per-core UV log tables ---------------------------------------
    x64 = np.clip(Xnew.astype(np.float64), 1e-30, None)
    u64 = np.log(x64)                                    # (N, D)
    v64 = np.log1p(-np.minimum(Xnew.astype(np.float64), 1.0 - 1e-15))
    split = _bf16_split if a16 == 2 else _fp22_split
    uv_np_dt = ml_dtypes.bfloat16 if a16 == 2 else np.float32
    uh, ul = split(u64)
    vh, vl = split(v64)
    uv_full = np.concatenate(
        [uh.T[None], vh.T[None], ul.T[None], vl.T[None]], axis=0
    )  # (4, D, N)
    uv_shards = []
    for i in range(NCORES):
        sl = uv_full[:, :, i * NLOC:(i + 1) * NLOC]      # (4, D, NLOC)
        uv_shards.append(np.ascontiguousarray(sl.reshape(4 * D, NLOC), uv_np_dt))

    # --- A selection/coefficient matrices (D*G, 128, RP) --------------
    ks = np.arange(K, dtype=np.float64)
    amat = np.zeros((D * G, 4 * D, RP), np.float64)
    for d in range(D):
        for g in range(G):
            A = amat[d * G + g]
            for c in range(CPG):
                p = g * CPG + c
                if p >= P:
                    continue
                col = perm[p, d]
                j = slice(K * c, K * c + K)
                A[col, j] = ks
                A[D + col, j] = ORDER - ks
                A[2 * D + col, j] = ks
                A[3 * D + col, j] = ORDER - ks
    amat = amat.astype(ml_dtypes.bfloat16) if a16 else amat.astype(np.float32)

    # --- block-diagonal chain weights ---------------------------------
    sc2 = prior_sc[:, 0] ** 2                            # (K,)
    wmean = np.zeros(((D - 1) * G, RP, RP), np.float64)
    wvar = np.zeros(((D - 1) * G, RP, RP), np.float64)
    for d in range(1, D):
        for g in range(G):
            Wm = wmean[(d - 1) * G + g]
            Wv = wvar[(d - 1) * G + g]
            for c in range(CPG):
                p = g * CPG + c
                if p >= P:
                    continue
                blk = slice(K * c, K * c + K)
                m = meanw[d - 1, p]                      # (K, K) [k, j]
                v = np.exp(varw[d - 1, p]) * sc2[None, :]
                if d == 1:
                    m = meanw0[p, 0][:, None] * m
                    v = (np.exp(varw0[p, 0]) * sc2)[:, None] * v
                Wm[blk, blk] = m
                Wv[blk, blk] = v
    if w16:
        wmh, wml = _bf16_split(wmean)
        wvh, wvl = _bf16_split(wvar)
        wmean = np.stack([wmh, wml], axis=1)             # (93, 2, RP, RP)
        wvar = np.stack([wvh, wvl], axis=1)
    else:
        wmean = _fp22_round(wmean)
        wvar = _fp22_round(wvar)

    # --- reduction vectors (G, RP, 2): col0 mean ones, col1 var 1/pp --
    # factor the geometric-mean scale of 1/post_prec out to the host so the
    # device-side values are ~1 (exactly 1 for uniform post_prec: no rounding)
    qbar = float(np.exp(np.mean(np.log(1.0 / post_prec))))
    qbar_inv = (1.0 / post_prec) / qbar
    redw = np.zeros((G, RP, 2), np.float64)
    for g in range(G):
        for c in range(CPG):
            p = g * CPG + c
            if p >= P:
                continue
            blk = slice(K * c, K * c + K)
            redw[g, blk, 0] = 1.0
            redw[g, blk, 1] = qbar_inv[p]
    redw = _fp22_round(redw)

    # --- exp biases: log binom / 2 log binom (per partition) ----------
    logb = np.log(np.array([comb(ORDER, k) for k in range(K)], np.float64))
    biasv = np.zeros((RP, 2), np.float64)
    biasv[:R, 0] = np.tile(logb, CPG)
    biasv[:R, 1] = 2.0 * np.tile(logb, CPG)
    biasv = biasv.astype(np.float32)

    shared = dict(amat=amat, wmean=wmean, wvar=wvar, redw=redw, biasv=biasv)
    return uv_shards, shared, qbar


def _build_module(nloc=NLOC):
    a16, w16, gp3, mulmod = _flags()
    nchunk = max(1, nloc // CHUNK)
    chunk = min(CHUNK, nloc)
    nred = max(1, nloc // HALF)
    rhalf = min(HALF, nloc)
    nh = chunk // rhalf                     # 512-halves per chunk

    A_DT = BF16 if a16 else F32R
    UV_DT = BF16 if a16 == 2 else F32R
    W_DT = BF16 if w16 else F32R
    wshape = [2, RP, RP] if w16 else [RP, RP]

    nc = bacc.Bacc("TRN2", target_bir_lowering=False, debug=False)
    uv_d = nc.dram_tensor("uv", [4 * D, nloc], UV_DT, kind="ExternalInput").ap()
    amat_d = nc.dram_tensor("amat", [D * G, 4 * D, RP], A_DT, kind="ExternalInput").ap()
    wm_d = nc.dram_tensor("wmean", [(D - 1) * G] + wshape, W_DT, kind="ExternalInput").ap()
    wv_d = nc.dram_tensor("wvar", [(D - 1) * G] + wshape, W_DT, kind="ExternalInput").ap()
    red_d = nc.dram_tensor("redw", [G, RP, 2], F32R, kind="ExternalInput").ap()
    bias_d = nc.dram_tensor("biasv", [RP, 2], F32, kind="ExternalInput").ap()
    out_d = nc.dram_tensor("out", [2, nloc], F32, kind="ExternalOutput").ap()

    tiles = [(d, g, ci) for d in range(D) for g in range(G) for ci in range(nchunk)]
    ntile = len(tiles)

    with tile.TileContext(nc) as tc:
        with (
            tc.tile_pool(name="persist", bufs=1) as persist,
            tc.tile_pool(name="wpool", bufs=4) as wpool,
            tc.tile_pool(name="mpool", bufs=4) as mpool,
            tc.tile_pool(name="psL", bufs=2, space="PSUM") as psL,
            tc.tile_pool(name="psC", bufs=2, space="PSUM") as psC,
        ):
            uv = persist.tile([4 * D, nloc], UV_DT, tag="uv")
            nc.sync.dma_start(uv[:], uv_d)
            bias = persist.tile([RP, 2], F32, tag="bias")
            nc.sync.dma_start(bias[:], bias_d)
            states = []
            for g in range(G):
                s = persist.tile([RP, nchunk, 2, chunk], F32R, tag=f"S{g}")
                states.append(s)
            redt = []
            for g in range(G):
                r = persist.tile([RP, 2], F32R, tag=f"RW{g}")
                nc.sync.dma_start(r[:], red_d[g])
                redt.append(r)

            loaded = {}

            def ensure_dg(t):
                if t >= ntile:
                    return
                d, g, _ = tiles[t]
                dg = d * G + g
                if dg in loaded:
                    return
                a_t = wpool.tile([4 * D, RP], A_DT, tag="A")
                nc.sync.dma_start(a_t[:], amat_d[dg])
                entry = {"A": a_t}
                if d >= 1:
                    wm_t = wpool.tile(wshape, W_DT, tag="WM")
                    nc.sync.dma_start(wm_t[:], wm_d[(d - 1) * G + g])
                    wv_t = wpool.tile(wshape, W_DT, tag="WV")
                    nc.sync.dma_start(wv_t[:], wv_d[(d - 1) * G + g])
                    entry["WM"] = wm_t
                    entry["WV"] = wv_t
                loaded[dg] = entry

            pstore = {}

            def emit_gather(t):
                d, g, ci = tiles[t]
                a_t = loaded[d * G + g]["A"]
                ps = psL.tile([RP, chunk], F32, tag="L")
                pstore[t] = ps
                c0 = ci * chunk
                for h in range(nh):
                    nc.tensor.matmul(
                        ps[:, h * rhalf:(h + 1) * rhalf],
                        a_t[:],
                        uv[:, c0 + h * rhalf:c0 + (h + 1) * rhalf],
                        start=True, stop=True)

            def emit_compute(t):
                d, g, ci = tiles[t]
                ps = pstore.pop(t)
                S = states[g]
                use_gp = (t % 3) < gp3
                if d == 0:
                    # initial states are the multipliers themselves
                    # (meanw0 / varw0 prefactors folded into d=1 weights)
                    nc.scalar.activation(
                        S[:, ci, 0, :], ps[:], EXP,
                        bias=bias[:, 0:1], scale=1.0)
                    if use_gp:
                        nc.gpsimd.tensor_tensor(
                            S[:, ci, 1, :], S[:, ci, 0, :], S[:, ci, 0, :], MULT)
                    else:
                        nc.scalar.activation(
                            S[:, ci, 1, :], ps[:], EXP,
                            bias=bias[:, 1:2], scale=2.0)
                    return
                ent = loaded[d * G + g]
                m_t = mpool.tile([RP, 2, chunk], F32, tag="M")
                nc.scalar.activation(
                    m_t[:, 0, :], ps[:], EXP, bias=bias[:, 0:1], scale=1.0)
                if use_gp:
                    nc.gpsimd.tensor_tensor(
                        m_t[:, 1, :], m_t[:, 0, :], m_t[:, 0, :], MULT)
                else:
                    nc.scalar.activation(
                        m_t[:, 1, :], ps[:], EXP, bias=bias[:, 1:2], scale=2.0)
                # chain matmuls live in their own small PSUM tiles so they
                # never wait on the exps; the DVE mul is the only consumer
                # of both streams
                c0 = ci * chunk
                for h in range(nh):
                    hs = slice(h * rhalf, (h + 1) * rhalf)
                    pc = psC.tile([RP, 2, rhalf], F32, tag="C")
                    for trow, wkey in ((1, "WV"), (0, "WM")):
                        w_t = ent[wkey]
                        dst = pc[:, trow, :]
                        src = S[:, ci, trow, hs]
                        if w16:
                            nc.tensor.matmul(dst, w_t[0], src, start=True, stop=False)
                            nc.tensor.matmul(dst, w_t[1], src, start=False, stop=True)
                        else:
                            nc.tensor.matmul(dst, w_t[:], src, start=True, stop=True)
                    if mulmod and (t * nh + h) % mulmod == 0:
                        # offload this multiply: ACT evacuates the chain
                        # PSUM to SBUF, GPSIMD does the multiply (GPSIMD
                        # cannot read PSUM directly)
                        sb = mpool.tile([RP, 2, rhalf], F32, tag="B")
                        nc.scalar.copy(sb[:], pc[:])
                        nc.gpsimd.tensor_tensor(
                            S[:, ci, :, hs], sb[:], m_t[:, :, hs], MULT)
                    else:
                        nc.vector.tensor_tensor(
                            S[:, ci, :, hs], pc[:], m_t[:, :, hs], MULT)

            # software-pipelined emission: gather one tile ahead
            ensure_dg(0)
            emit_gather(0)
            for t in range(ntile):
                ensure_dg(t + 1)
                ensure_dg(t + nchunk + 1)    # prefetch next (d,g) weights
                if t + 1 < ntile:
                    emit_gather(t + 1)
                emit_compute(t)

            # ---- final reduction: sum over (chain, k) partitions -----
            # single partition row: [mean(nloc) | var(nloc)] (engine APs
            # must start on quadrant-aligned partitions, so no row 1)
            outs = persist.tile([1, 2 * nloc], F32, tag="outs")
            for ci in range(nred):
                o0 = ci * rhalf
                cc, off = divmod(o0, chunk)
                pr = psC.tile([1, 2, rhalf], F32, tag="C")
                for g in range(G):
                    nc.tensor.matmul(
                        pr[:, 0, :], redt[g][:, 0:1],
                        states[g][:, cc, 0, off:off + rhalf],
                        start=(g == 0), stop=(g == G - 1))
                for g in range(G):
                    nc.tensor.matmul(
                        pr[:, 1, :], redt[g][:, 1:2],
                        states[g][:, cc, 1, off:off + rhalf],
                        start=(g == 0), stop=(g == G - 1))
                nc.scalar.copy(outs[0:1, o0:o0 + rhalf], pr[:, 0, :])
                nc.scalar.copy(
                    outs[0:1, nloc + o0:nloc + o0 + rhalf], pr[:, 1, :])
            nc.sync.dma_start(out_d.rearrange("a b -> (a b)")[None, :], outs[:])

    nc.compile()
    return nc


def kernel(Xnew, meanw0, meanw, varw0, varw, prior_sc, post_prec, perm):
    uv_shards, shared, qbar = _host_tensors(
        Xnew, meanw0, meanw, varw0, varw, prior_sc, post_prec, perm)
    nc = _build_module(NLOC)
    in_maps = [dict(uv=uv_shards[i], **shared) for i in range(NCORES)]
    res = bass_utils.run_bass_kernel_spmd(
        nc, in_maps, core_ids=list(range(NCORES)))
    outs = [res.results[i]["out"] for i in range(NCORES)]
    f_mean = np.concatenate([o[0] for o in outs]).reshape(N, 1).astype(np.float32)
    f_var = (np.concatenate([o[1] for o in outs]).reshape(N, 1)
             * np.float32(qbar)).astype(np.float32)
    return f_mean, f_var


# revision 14
# speedup vs baseline: 1.1308x; 1.1308x over previous
"""Trainium2 Bass kernel for nn_BezierButtress (Bernstein-basis permutation chains).

Math (per permutation chain p, over depth d = 0..31):
    S_mean <- (S_mean @ Wm_d) * B(x_{perm[p,d]})        (K=17 wide state)
    S_var  <- (S_var  @ Wv_d) * B(x_{perm[p,d]})^2
    outputs: f_mean[n] = sum_{p,k} S_mean, f_var[n] = sum_{p,k} S_var / post_prec[p]

Device strategy (data-parallel over N across 8 cores, 3072 rows each):
  * state layout: (7 chains x 17 k -> 128 partitions incl. pad, n free),
    block-diagonal 128x128 fp32r chain matmuls (3 groups cover 20 chains).
  * per-step Bernstein multipliers built in log space: one PE matmul contracts
    a baked selection/coefficient matrix A_{d,g} (128 x 128) against a resident
    log-table UV (U_hi/V_hi/U_lo/V_lo, 128 x n) giving
    logM = k*log(x_c) + (16-k)*log(1-x_c) exactly (hi/lo splitting cancels the
    PE fp22 truncation); then ACT computes exp(logM + log binom); the squared
    multiplier comes from exp(scale=2) on ACT or an SBUF square on GPSIMD.
  * meanw0 / exp(varw0)*sc2 / sc2 column scale / 1/post_prec are all folded
    host-side into the baked block-diagonal weights & reduction vectors
    (weights pre-rounded to e10m11 so the PE fp32r truncation is a no-op).
  * emission is software-pipelined one tile ahead (gather of tile t+1 before
    compute of tile t) over a double-buffered 4-bank PSUM tile whose banks are
    reused logM -> chain outputs within each tile's lifetime.
"""

import os
import numpy as np
import ml_dtypes
from math import comb

import concourse.bass as bass
import concourse.mybir as mybir
import concourse.tile as tile
from concourse import bacc
from concourse import bass_utils

ORDER = 16
K = 17
D = 32
P = 20
N = 24576
NCORES = 8
NLOC = N // NCORES        # 3072
CPG = 7                   # chain slots per group
G = 3                     # groups (7, 7, 6 + 1 pad)
R = CPG * K               # 119 active partitions
RP = 128                  # padded partition count
CHUNK = 1024
HALF = 512
F32 = mybir.dt.float32
F32R = mybir.dt.float32r
BF16 = mybir.dt.bfloat16
EXP = mybir.ActivationFunctionType.Exp
MULT = mybir.AluOpType.mult


def _flags():
    # NOTE: walrus rejects mixed 32-bit/16-bit matmul inputs (NCC_IBIR034),
    # so bf16 operands require BOTH sides bf16.  a16=2 runs the whole gather
    # matmul (A and the UV log-table) in bf16 -- A entries are small exact
    # integers and UV is hi/lo split, so the effective log-table mantissa is
    # ~16 bits; bf16 streams ~2x faster through the PE than fp32r.
    a16 = int(os.environ.get("BB_A16", "0"))         # 0=f32r, 2=bf16 gather
    w16 = bool(int(os.environ.get("BB_W16", "0")))   # bf16 hi/lo weights (invalid)
    gp3 = int(os.environ.get("BB_GP3", "2"))         # GP square 2-of-3 tiles
    mulmod = int(os.environ.get("BB_MULMOD", "0"))   # 1-in-mulmod muls via ACT+GP
    return a16, w16, gp3, mulmod


def _fp22_round(x64):
    """Round float64 to the nearest fp22 (e10m11) value, returned as float32.
    The PE's fp32r path *truncates* inputs to fp22; feeding it pre-rounded
    values makes that truncation a no-op and kills the systematic bias."""
    x32 = x64.astype(np.float32)
    u = x32.view(np.uint32).astype(np.uint64)
    u = ((u + 0x800) & 0xFFFFF000).astype(np.uint32)   # round-half-up on m11
    return u.view(np.float32)


def _fp22_split(x64):
    """Split float64 -> (hi, lo) float32 with hi exactly representable in
    fp22 (e10m11), so PE fp32r matmuls consume hi/lo exactly."""
    x32 = x64.astype(np.float32)
    hi = (x32.view(np.uint32) & np.uint32(0xFFFFF000)).view(np.float32)
    lo = (x64 - hi.astype(np.float64)).astype(np.float32)
    return hi, lo


def _bf16_split(x64):
    hi = x64.astype(ml_dtypes.bfloat16)
    lo = (x64 - hi.astype(np.float64)).astype(ml_dtypes.bfloat16)
    return hi, lo


def _host_tensors(Xnew, meanw0, meanw, varw0, varw, prior_sc, post_prec, perm):
    a16, w16, _, _ = _flags()
    Xnew = np.asarray(Xnew, np.float32)
    meanw0 = np.asarray(meanw0, np.float64)   # (P, 1, K)
    meanw = np.asarray(meanw, np.float64)     # (D-1, P, K, K)
    varw0 = np.asarray(varw0, np.float64)     # (P, 1, K)
    varw = np.asarray(varw, np.float64)       # (D-1, P, K, K)
    prior_sc = np.asarray(prior_sc, np.float64)  # (K, 1)
    post_prec = np.asarray(post_prec, np.float64)  # (P,)
    perm = np.asarray(perm)                   # (P, D) int

    # --- per-core UV log tables ---------------------------------------
    x64 = np.clip(Xnew.astype(np.float64), 1e-30, None)
    u64 = np.log(x64)                                    # (N, D)
    v64 = np.log1p(-np.minimum(Xnew.astype(np.float64), 1.0 - 1e-15))
    split = _bf16_split if a16 == 2 else _fp22_split
    uv_np_dt = ml_dtypes.bfloat16 if a16 == 2 else np.float32
    uh, ul = split(u64)
    vh, vl = split(v64)
    uv_full = np.concatenate(
        [uh.T[None], vh.T[None], ul.T[None], vl.T[None]], axis=0
    )  # (4, D, N)
    uv_shards = []
    for i in range(NCORES):
        sl = uv_full[:, :, i * NLOC:(i + 1) * NLOC]      # (4, D, NLOC)
        uv_shards.append(np.ascontiguousarray(sl.reshape(4 * D, NLOC), uv_np_dt))

    # --- A selection/coefficient matrices (D*G, 128, RP) --------------
    ks = np.arange(K, dtype=np.float64)
    amat = np.zeros((D * G, 4 * D, RP), np.float64)
    for d in range(D):
        for g in range(G):
            A = amat[d * G + g]
            for c in range(CPG):
                p = g * CPG + c
                if p >= P:
                    continue
                col = perm[p, d]
                j = slice(K * c, K * c + K)
                A[col, j] = ks
                A[D + col, j] = ORDER - ks
                A[2 * D + col, j] = ks
                A[3 * D + col, j] = ORDER - ks
    amat = amat.astype(ml_dtypes.bfloat16) if a16 else amat.astype(np.float32)

    # --- block-diagonal chain weights ---------------------------------
    sc2 = prior_sc[:, 0] ** 2                            # (K,)
    wmean = np.zeros(((D - 1) * G, RP, RP), np.float64)
    wvar = np.zeros(((D - 1) * G, RP, RP), np.float64)
    for d in range(1, D):
        for g in range(G):
            Wm = wmean[(d - 1) * G + g]
            Wv = wvar[(d - 1) * G + g]
            for c in range(CPG):
                p = g * CPG + c
                if p >= P:
                    continue
                blk = slice(K * c, K * c + K)
                m = meanw[d - 1, p]                      # (K, K) [k, j]
                v = np.exp(varw[d - 1, p]) * sc2[None, :]
                if d == 1:
                    m = meanw0[p, 0][:, None] * m
                    v = (np.exp(varw0[p, 0]) * sc2)[:, None] * v
                Wm[blk, blk] = m
                Wv[blk, blk] = v
    if w16:
        wmh, wml = _bf16_split(wmean)
        wvh, wvl = _bf16_split(wvar)
        wmean = np.stack([wmh, wml], axis=1)             # (93, 2, RP, RP)
        wvar = np.stack([wvh, wvl], axis=1)
    else:
        wmean = _fp22_round(wmean)
        wvar = _fp22_round(wvar)

    # --- reduction vectors (G, RP, 2): col0 mean ones, col1 var 1/pp --
    # factor the geometric-mean scale of 1/post_prec out to the host so the
    # device-side values are ~1 (exactly 1 for uniform post_prec: no rounding)
    qbar = float(np.exp(np.mean(np.log(1.0 / post_prec))))
    qbar_inv = (1.0 / post_prec) / qbar
    redw = np.zeros((G, RP, 2), np.float64)
    for g in range(G):
        for c in range(CPG):
            p = g * CPG + c
            if p >= P:
                continue
            blk = slice(K * c, K * c + K)
            redw[g, blk, 0] = 1.0
            redw[g, blk, 1] = qbar_inv[p]
    redw = _fp22_round(redw)

    # --- exp biases: log binom / 2 log binom (per partition) ----------
    logb = np.log(np.array([comb(ORDER, k) for k in range(K)], np.float64))
    biasv = np.zeros((RP, 2), np.float64)
    biasv[:R, 0] = np.tile(logb, CPG)
    biasv[:R, 1] = 2.0 * np.tile(logb, CPG)
    biasv = biasv.astype(np.float32)

    shared = dict(amat=amat, wmean=wmean, wvar=wvar, redw=redw, biasv=biasv)
    return uv_shards, shared, qbar


def _build_module(nloc=NLOC):
    a16, w16, gp3, mulmod = _flags()
    nchunk = max(1, nloc // CHUNK)
    chunk = min(CHUNK, nloc)
    nred = max(1, nloc // HALF)
    rhalf = min(HALF, nloc)
    nh = chunk // rhalf                     # 512-halves per chunk

    A_DT = BF16 if a16 else F32R
    UV_DT = BF16 if a16 == 2 else F32R
    W_DT = BF16 if w16 else F32R
    wshape = [2, RP, RP] if w16 else [RP, RP]

    nc = bacc.Bacc("TRN2", target_bir_lowering=False, debug=False)
    uv_d = nc.dram_tensor("uv", [4 * D, nloc], UV_DT, kind="ExternalInput").ap()
    amat_d = nc.dram_tensor("amat", [D * G, 4 * D, RP], A_DT, kind="ExternalInput").ap()
    wm_d = nc.dram_tensor("wmean", [(D - 1) * G] + wshape, W_DT, kind="ExternalInput").ap()
    wv_d = nc.dram_tensor("wvar", [(D - 1) * G] + wshape, W_DT, kind="ExternalInput").ap()
    red_d = nc.dram_tensor("redw", [G, RP, 2], F32R, kind="ExternalInput").ap()
    bias_d = nc.dram_tensor("biasv", [RP, 2], F32, kind="ExternalInput").ap()
    out_d = nc.dram_tensor("out", [2, nloc], F32, kind="ExternalOutput").ap()

    tiles = [(d, g, ci) for d in range(D) for g in range(G) for ci in range(nchunk)]
    ntile = len(tiles)

    with tile.TileContext(nc) as tc:
        with (
            tc.tile_pool(name="persist", bufs=1) as persist,
            tc.tile_pool(name="wpool", bufs=6) as wpool,
            tc.tile_pool(name="mpool", bufs=6) as mpool,
            tc.tile_pool(name="psL", bufs=2, space="PSUM") as psL,
            tc.tile_pool(name="psC", bufs=2, space="PSUM") as psC,
        ):
            uv = persist.tile([4 * D, nloc], UV_DT, tag="uv")
            nc.sync.dma_start(uv[:], uv_d)
            bias = persist.tile([RP, 2], F32, tag="bias")
            nc.sync.dma_start(bias[:], bias_d)
            states = []
            for g in range(G):
                s = persist.tile([RP, nchunk, 2, chunk], F32R, tag=f"S{g}")
                states.append(s)
            redt = []
            for g in range(G):
                r = persist.tile([RP, 2], F32R, tag=f"RW{g}")
                nc.sync.dma_start(r[:], red_d[g])
                redt.append(r)

            loaded = {}

            def ensure_dg(t):
                if t >= ntile:
                    return
                d, g, _ = tiles[t]
                dg = d * G + g
                if dg in loaded:
                    return
                a_t = wpool.tile([4 * D, RP], A_DT, tag="A")
                nc.sync.dma_start(a_t[:], amat_d[dg])
                entry = {"A": a_t}
                if d >= 1:
                    wm_t = wpool.tile(wshape, W_DT, tag="WM")
                    nc.sync.dma_start(wm_t[:], wm_d[(d - 1) * G + g])
                    wv_t = wpool.tile(wshape, W_DT, tag="WV")
                    nc.sync.dma_start(wv_t[:], wv_d[(d - 1) * G + g])
                    entry["WM"] = wm_t
                    entry["WV"] = wv_t
                loaded[dg] = entry

            pstore = {}

            def emit_gather(t):
                d, g, ci = tiles[t]
                a_t = loaded[d * G + g]["A"]
                ps = psL.tile([RP, chunk], F32, tag="L")
                pstore[t] = ps
                c0 = ci * chunk
                for h in range(nh):
                    nc.tensor.matmul(
                        ps[:, h * rhalf:(h + 1) * rhalf],
                        a_t[:],
                        uv[:, c0 + h * rhalf:c0 + (h + 1) * rhalf],
                        start=True, stop=True)

            def emit_compute(t):
                d, g, ci = tiles[t]
                ps = pstore.pop(t)
                S = states[g]
                use_gp = (t % 3) < gp3
                if d == 0:
                    # initial states are the multipliers themselves
                    # (meanw0 / varw0 prefactors folded into d=1 weights)
                    nc.scalar.activation(
                        S[:, ci, 0, :], ps[:], EXP,
                        bias=bias[:, 0:1], scale=1.0)
                    if use_gp:
                        nc.gpsimd.tensor_tensor(
                            S[:, ci, 1, :], S[:, ci, 0, :], S[:, ci, 0, :], MULT)
                    else:
                        nc.scalar.activation(
                            S[:, ci, 1, :], ps[:], EXP,
                            bias=bias[:, 1:2], scale=2.0)
                    return
                ent = loaded[d * G + g]
                m_t = mpool.tile([RP, 2, chunk], F32, tag="M")
                nc.scalar.activation(
                    m_t[:, 0, :], ps[:], EXP, bias=bias[:, 0:1], scale=1.0)
                if use_gp:
                    nc.gpsimd.tensor_tensor(
                        m_t[:, 1, :], m_t[:, 0, :], m_t[:, 0, :], MULT)
                else:
                    nc.scalar.activation(
                        m_t[:, 1, :], ps[:], EXP, bias=bias[:, 1:2], scale=2.0)
                # chain matmuls live in their own small PSUM tiles so they
                # never wait on the exps; the DVE mul is the only consumer
                # of both streams
                c0 = ci * chunk
                for h in range(nh):
                    hs = slice(h * rhalf, (h + 1) * rhalf)
                    pc = psC.tile([RP, 2, rhalf], F32, tag="C")
                    for trow, wkey in ((1, "WV"), (0, "WM")):
                        w_t = ent[wkey]
                        dst = pc[:, trow, :]
                        src = S[:, ci, trow, hs]
                        if w16:
                            nc.tensor.matmul(dst, w_t[0], src, start=True, stop=False)
                            nc.tensor.matmul(dst, w_t[1], src, start=False, stop=True)
                        else:
                            nc.tensor.matmul(dst, w_t[:], src, start=True, stop=True)
                    if mulmod and (t * nh + h) % mulmod == 0:
                        # offload this multiply: ACT evacuates the chain
                        # PSUM to SBUF, GPSIMD does the multiply (GPSIMD
                        # cannot read PSUM directly)
                        sb = mpool.tile([RP, 2, rhalf], F32, tag="B")
                        nc.scalar.copy(sb[:], pc[:])
                        nc.gpsimd.tensor_tensor(
                            S[:, ci, :, hs], sb[:], m_t[:, :, hs], MULT)
                    else:
                        nc.vector.tensor_tensor(
                            S[:, ci, :, hs], pc[:], m_t[:, :, hs], MULT)

            # software-pipelined emission: gather one tile ahead
            ensure_dg(0)
            emit_gather(0)
            for t in range(ntile):
                ensure_dg(t + 1)
                ensure_dg(t + nchunk + 1)    # prefetch next (d,g) weights
                if t + 1 < ntile:
                    emit_gather(t + 1)
                emit_compute(t)

            # ---- final reduction: sum over (chain, k) partitions -----
            # single partition row: [mean(nloc) | var(nloc)] (engine APs
            # must start on quadrant-aligned partitions, so no row 1)
            outs = persist.tile([1, 2 * nloc], F32, tag="outs")
            for ci in range(nred):
                o0 = ci * rhalf
                cc, off = divmod(o0, chunk)
                pr = psC.tile([1, 2, rhalf], F32, tag="C")
                for g in range(G):
                    nc.tensor.matmul(
                        pr[:, 0, :], redt[g][:, 0:1],
                        states[g][:, cc, 0, off:off + rhalf],
                        start=(g == 0), stop=(g == G - 1))
                for g in range(G):
                    nc.tensor.matmul(
                        pr[:, 1, :], redt[g][:, 1:2],
                        states[g][:, cc, 1, off:off + rhalf],
                        start=(g == 0), stop=(g == G - 1))
                nc.scalar.copy(outs[0:1, o0:o0 + rhalf], pr[:, 0, :])
                nc.scalar.copy(
                    outs[0:1, nloc + o0:nloc + o0 + rhalf], pr[:, 1, :])
            nc.sync.dma_start(out_d.rearrange("a b -> (a b)")[None, :], outs[:])

    nc.compile()
    return nc


def kernel(Xnew, meanw0, meanw, varw0, varw, prior_sc, post_prec, perm):
    uv_shards, shared, qbar = _host_tensors(
        Xnew, meanw0, meanw, varw0, varw, prior_sc, post_prec, perm)
    nc = _build_module(NLOC)
    in_maps = [dict(uv=uv_shards[i], **shared) for i in range(NCORES)]
    res = bass_utils.run_bass_kernel_spmd(
        nc, in_maps, core_ids=list(range(NCORES)))
    outs = [res.results[i]["out"] for i in range(NCORES)]
    f_mean = np.concatenate([o[0] for o in outs]).reshape(N, 1).astype(np.float32)
    f_var = (np.concatenate([o[1] for o in outs]).reshape(N, 1)
             * np.float32(qbar)).astype(np.float32)
    return f_mean, f_var


# revision 15
# speedup vs baseline: 1.1349x; 1.0036x over previous
"""Trainium2 Bass kernel for nn_BezierButtress (Bernstein-basis permutation chains).

Math (per permutation chain p, over depth d = 0..31):
    S_mean <- (S_mean @ Wm_d) * B(x_{perm[p,d]})        (K=17 wide state)
    S_var  <- (S_var  @ Wv_d) * B(x_{perm[p,d]})^2
    outputs: f_mean[n] = sum_{p,k} S_mean, f_var[n] = sum_{p,k} S_var / post_prec[p]

Device strategy (data-parallel over N across 8 cores, 3072 rows each):
  * state layout: (7 chains x 17 k -> 128 partitions incl. pad, n free),
    block-diagonal 128x128 fp32r chain matmuls (3 groups cover 20 chains).
  * per-step Bernstein multipliers built in log space: one PE matmul contracts
    a baked selection/coefficient matrix A_{d,g} (128 x 128) against a resident
    log-table UV (U_hi/V_hi/U_lo/V_lo, 128 x n) giving
    logM = k*log(x_c) + (16-k)*log(1-x_c) exactly (hi/lo splitting cancels the
    PE fp22 truncation); then ACT computes exp(logM + log binom); the squared
    multiplier comes from exp(scale=2) on ACT or an SBUF square on GPSIMD.
  * meanw0 / exp(varw0)*sc2 / sc2 column scale / 1/post_prec are all folded
    host-side into the baked block-diagonal weights & reduction vectors
    (weights pre-rounded to e10m11 so the PE fp32r truncation is a no-op).
  * emission is software-pipelined one tile ahead (gather of tile t+1 before
    compute of tile t) over a double-buffered 4-bank PSUM tile whose banks are
    reused logM -> chain outputs within each tile's lifetime.
"""

import os
import numpy as np
import ml_dtypes
from math import comb

import concourse.bass as bass
import concourse.mybir as mybir
import concourse.tile as tile
from concourse import bacc
from concourse import bass_utils

ORDER = 16
K = 17
D = 32
P = 20
N = 24576
NCORES = 8
NLOC = N // NCORES        # 3072
CPG = 7                   # chain slots per group
G = 3                     # groups (7, 7, 6 + 1 pad)
R = CPG * K               # 119 active partitions
RP = 128                  # padded partition count
CHUNK = 1024
HALF = 512
F32 = mybir.dt.float32
F32R = mybir.dt.float32r
BF16 = mybir.dt.bfloat16
EXP = mybir.ActivationFunctionType.Exp
MULT = mybir.AluOpType.mult


def _flags():
    # NOTE: walrus rejects mixed 32-bit/16-bit matmul inputs (NCC_IBIR034),
    # so bf16 operands require BOTH sides bf16.  a16=2 runs the whole gather
    # matmul (A and the UV log-table) in bf16 -- A entries are small exact
    # integers and UV is hi/lo split, so the effective log-table mantissa is
    # ~16 bits; bf16 streams ~2x faster through the PE than fp32r.
    a16 = int(os.environ.get("BB_A16", "0"))         # 0=f32r, 2=bf16 gather
    w16 = bool(int(os.environ.get("BB_W16", "0")))   # bf16 hi/lo weights (invalid)
    gp3 = int(os.environ.get("BB_GP3", "2"))         # GP square 2-of-3 tiles
    mulmod = int(os.environ.get("BB_MULMOD", "0"))   # 1-in-mulmod muls via ACT+GP
    return a16, w16, gp3, mulmod


def _fp22_round(x64):
    """Round float64 to the nearest fp22 (e10m11) value, returned as float32.
    The PE's fp32r path *truncates* inputs to fp22; feeding it pre-rounded
    values makes that truncation a no-op and kills the systematic bias."""
    x32 = x64.astype(np.float32)
    u = x32.view(np.uint32).astype(np.uint64)
    u = ((u + 0x800) & 0xFFFFF000).astype(np.uint32)   # round-half-up on m11
    return u.view(np.float32)


def _fp22_split(x64):
    """Split float64 -> (hi, lo) float32 with hi exactly representable in
    fp22 (e10m11), so PE fp32r matmuls consume hi/lo exactly."""
    x32 = x64.astype(np.float32)
    hi = (x32.view(np.uint32) & np.uint32(0xFFFFF000)).view(np.float32)
    lo = (x64 - hi.astype(np.float64)).astype(np.float32)
    return hi, lo


def _bf16_split(x64):
    hi = x64.astype(ml_dtypes.bfloat16)
    lo = (x64 - hi.astype(np.float64)).astype(ml_dtypes.bfloat16)
    return hi, lo


def _host_tensors(Xnew, meanw0, meanw, varw0, varw, prior_sc, post_prec, perm):
    a16, w16, _, _ = _flags()
    Xnew = np.asarray(Xnew, np.float32)
    meanw0 = np.asarray(meanw0, np.float64)   # (P, 1, K)
    meanw = np.asarray(meanw, np.float64)     # (D-1, P, K, K)
    varw0 = np.asarray(varw0, np.float64)     # (P, 1, K)
    varw = np.asarray(varw, np.float64)       # (D-1, P, K, K)
    prior_sc = np.asarray(prior_sc, np.float64)  # (K, 1)
    post_prec = np.asarray(post_prec, np.float64)  # (P,)
    perm = np.asarray(perm)                   # (P, D) int

    # --- per-core UV log tables ---------------------------------------
    x64 = np.clip(Xnew.astype(np.float64), 1e-30, None)
    u64 = np.log(x64)                                    # (N, D)
    v64 = np.log1p(-np.minimum(Xnew.astype(np.float64), 1.0 - 1e-15))
    split = _bf16_split if a16 == 2 else _fp22_split
    uv_np_dt = ml_dtypes.bfloat16 if a16 == 2 else np.float32
    uh, ul = split(u64)
    vh, vl = split(v64)
    uv_full = np.concatenate(
        [uh.T[None], vh.T[None], ul.T[None], vl.T[None]], axis=0
    )  # (4, D, N)
    uv_shards = []
    for i in range(NCORES):
        sl = uv_full[:, :, i * NLOC:(i + 1) * NLOC]      # (4, D, NLOC)
        uv_shards.append(np.ascontiguousarray(sl.reshape(4 * D, NLOC), uv_np_dt))

    # --- A selection/coefficient matrices (D*G, 128, RP) --------------
    ks = np.arange(K, dtype=np.float64)
    amat = np.zeros((D * G, 4 * D, RP), np.float64)
    for d in range(D):
        for g in range(G):
            A = amat[d * G + g]
            for c in range(CPG):
                p = g * CPG + c
                if p >= P:
                    continue
                col = perm[p, d]
                j = slice(K * c, K * c + K)
                A[col, j] = ks
                A[D + col, j] = ORDER - ks
                A[2 * D + col, j] = ks
                A[3 * D + col, j] = ORDER - ks
    amat = amat.astype(ml_dtypes.bfloat16) if a16 else amat.astype(np.float32)

    # --- block-diagonal chain weights ---------------------------------
    sc2 = prior_sc[:, 0] ** 2                            # (K,)
    wmean = np.zeros(((D - 1) * G, RP, RP), np.float64)
    wvar = np.zeros(((D - 1) * G, RP, RP), np.float64)
    for d in range(1, D):
        for g in range(G):
            Wm = wmean[(d - 1) * G + g]
            Wv = wvar[(d - 1) * G + g]
            for c in range(CPG):
                p = g * CPG + c
                if p >= P:
                    continue
                blk = slice(K * c, K * c + K)
                m = meanw[d - 1, p]                      # (K, K) [k, j]
                v = np.exp(varw[d - 1, p]) * sc2[None, :]
                if d == 1:
                    m = meanw0[p, 0][:, None] * m
                    v = (np.exp(varw0[p, 0]) * sc2)[:, None] * v
                Wm[blk, blk] = m
                Wv[blk, blk] = v
    if w16:
        wmh, wml = _bf16_split(wmean)
        wvh, wvl = _bf16_split(wvar)
        wmean = np.stack([wmh, wml], axis=1)             # (93, 2, RP, RP)
        wvar = np.stack([wvh, wvl], axis=1)
    else:
        wmean = _fp22_round(wmean)
        wvar = _fp22_round(wvar)

    # --- reduction vectors (G, RP, 2): col0 mean ones, col1 var 1/pp --
    # factor the geometric-mean scale of 1/post_prec out to the host so the
    # device-side values are ~1 (exactly 1 for uniform post_prec: no rounding)
    if np.all(post_prec > 0):
        qbar = float(np.exp(np.mean(np.log(1.0 / post_prec))))
    else:
        qbar = 1.0
    qbar_inv = (1.0 / post_prec) / qbar
    redw = np.zeros((G, RP, 2), np.float64)
    for g in range(G):
        for c in range(CPG):
            p = g * CPG + c
            if p >= P:
                continue
            blk = slice(K * c, K * c + K)
            redw[g, blk, 0] = 1.0
            redw[g, blk, 1] = qbar_inv[p]
    redw = _fp22_round(redw)

    # --- exp biases: log binom / 2 log binom (per partition) ----------
    logb = np.log(np.array([comb(ORDER, k) for k in range(K)], np.float64))
    biasv = np.zeros((RP, 2), np.float64)
    biasv[:R, 0] = np.tile(logb, CPG)
    biasv[:R, 1] = 2.0 * np.tile(logb, CPG)
    biasv = biasv.astype(np.float32)

    shared = dict(amat=amat, wmean=wmean, wvar=wvar, redw=redw, biasv=biasv)
    return uv_shards, shared, qbar


def _build_module(nloc=NLOC):
    a16, w16, gp3, mulmod = _flags()
    nchunk = max(1, nloc // CHUNK)
    chunk = min(CHUNK, nloc)
    nred = max(1, nloc // HALF)
    rhalf = min(HALF, nloc)
    nh = chunk // rhalf                     # 512-halves per chunk

    A_DT = BF16 if a16 else F32R
    UV_DT = BF16 if a16 == 2 else F32R
    W_DT = BF16 if w16 else F32R
    wshape = [2, RP, RP] if w16 else [RP, RP]

    nc = bacc.Bacc("TRN2", target_bir_lowering=False, debug=False)
    uv_d = nc.dram_tensor("uv", [4 * D, nloc], UV_DT, kind="ExternalInput").ap()
    amat_d = nc.dram_tensor("amat", [D * G, 4 * D, RP], A_DT, kind="ExternalInput").ap()
    wm_d = nc.dram_tensor("wmean", [(D - 1) * G] + wshape, W_DT, kind="ExternalInput").ap()
    wv_d = nc.dram_tensor("wvar", [(D - 1) * G] + wshape, W_DT, kind="ExternalInput").ap()
    red_d = nc.dram_tensor("redw", [G, RP, 2], F32R, kind="ExternalInput").ap()
    bias_d = nc.dram_tensor("biasv", [RP, 2], F32, kind="ExternalInput").ap()
    out_d = nc.dram_tensor("out", [2, nloc], F32, kind="ExternalOutput").ap()

    tiles = [(d, g, ci) for d in range(D) for g in range(G) for ci in range(nchunk)]
    ntile = len(tiles)

    with tile.TileContext(nc) as tc:
        with (
            tc.tile_pool(name="persist", bufs=1) as persist,
            tc.tile_pool(name="wpool", bufs=6) as wpool,
            tc.tile_pool(name="mpool", bufs=6) as mpool,
            tc.tile_pool(name="psL", bufs=2, space="PSUM") as psL,
            tc.tile_pool(name="psC", bufs=2, space="PSUM") as psC,
        ):
            uv = persist.tile([4 * D, nloc], UV_DT, tag="uv")
            nc.sync.dma_start(uv[:], uv_d)
            bias = persist.tile([RP, 2], F32, tag="bias")
            nc.sync.dma_start(bias[:], bias_d)
            states = []
            for g in range(G):
                s = persist.tile([RP, nchunk, 2, chunk], F32R, tag=f"S{g}")
                states.append(s)
            redt = []
            for g in range(G):
                r = persist.tile([RP, 2], F32R, tag=f"RW{g}")
                nc.sync.dma_start(r[:], red_d[g])
                redt.append(r)

            loaded = {}

            def ensure_dg(t):
                if t >= ntile:
                    return
                d, g, _ = tiles[t]
                dg = d * G + g
                if dg in loaded:
                    return
                a_t = wpool.tile([4 * D, RP], A_DT, tag="A")
                nc.sync.dma_start(a_t[:], amat_d[dg])
                entry = {"A": a_t}
                if d >= 1:
                    wm_t = wpool.tile(wshape, W_DT, tag="WM")
                    nc.sync.dma_start(wm_t[:], wm_d[(d - 1) * G + g])
                    wv_t = wpool.tile(wshape, W_DT, tag="WV")
                    nc.sync.dma_start(wv_t[:], wv_d[(d - 1) * G + g])
                    entry["WM"] = wm_t
                    entry["WV"] = wv_t
                loaded[dg] = entry

            pstore = {}

            def emit_gather(t):
                d, g, ci = tiles[t]
                a_t = loaded[d * G + g]["A"]
                ps = psL.tile([RP, chunk], F32, tag="L")
                pstore[t] = ps
                c0 = ci * chunk
                for h in range(nh):
                    nc.tensor.matmul(
                        ps[:, h * rhalf:(h + 1) * rhalf],
                        a_t[:],
                        uv[:, c0 + h * rhalf:c0 + (h + 1) * rhalf],
                        start=True, stop=True)

            def emit_compute(t):
                d, g, ci = tiles[t]
                ps = pstore.pop(t)
                S = states[g]
                use_gp = (t % 3) < gp3
                if d == 0:
                    # initial states are the multipliers themselves
                    # (meanw0 / varw0 prefactors folded into d=1 weights)
                    nc.scalar.activation(
                        S[:, ci, 0, :], ps[:], EXP,
                        bias=bias[:, 0:1], scale=1.0)
                    if use_gp:
                        nc.gpsimd.tensor_tensor(
                            S[:, ci, 1, :], S[:, ci, 0, :], S[:, ci, 0, :], MULT)
                    else:
                        nc.scalar.activation(
                            S[:, ci, 1, :], ps[:], EXP,
                            bias=bias[:, 1:2], scale=2.0)
                    return
                ent = loaded[d * G + g]
                m_t = mpool.tile([RP, 2, chunk], F32, tag="M")
                nc.scalar.activation(
                    m_t[:, 0, :], ps[:], EXP, bias=bias[:, 0:1], scale=1.0)
                if use_gp:
                    nc.gpsimd.tensor_tensor(
                        m_t[:, 1, :], m_t[:, 0, :], m_t[:, 0, :], MULT)
                else:
                    nc.scalar.activation(
                        m_t[:, 1, :], ps[:], EXP, bias=bias[:, 1:2], scale=2.0)
                # chain matmuls live in their own small PSUM tiles so they
                # never wait on the exps; the DVE mul is the only consumer
                # of both streams
                c0 = ci * chunk
                for h in range(nh):
                    hs = slice(h * rhalf, (h + 1) * rhalf)
                    pc = psC.tile([RP, 2, rhalf], F32, tag="C")
                    for trow, wkey in ((1, "WV"), (0, "WM")):
                        w_t = ent[wkey]
                        dst = pc[:, trow, :]
                        src = S[:, ci, trow, hs]
                        if w16:
                            nc.tensor.matmul(dst, w_t[0], src, start=True, stop=False)
                            nc.tensor.matmul(dst, w_t[1], src, start=False, stop=True)
                        else:
                            nc.tensor.matmul(dst, w_t[:], src, start=True, stop=True)
                    if mulmod and (t * nh + h) % mulmod == 0:
                        # offload this multiply: ACT evacuates the chain
                        # PSUM to SBUF, GPSIMD does the multiply (GPSIMD
                        # cannot read PSUM directly)
                        sb = mpool.tile([RP, 2, rhalf], F32, tag="B")
                        nc.scalar.copy(sb[:], pc[:])
                        nc.gpsimd.tensor_tensor(
                            S[:, ci, :, hs], sb[:], m_t[:, :, hs], MULT)
                    else:
                        nc.vector.tensor_tensor(
                            S[:, ci, :, hs], pc[:], m_t[:, :, hs], MULT)

            # software-pipelined emission: gather one tile ahead
            ensure_dg(0)
            emit_gather(0)
            for t in range(ntile):
                ensure_dg(t + 1)
                ensure_dg(t + nchunk + 1)    # prefetch next (d,g) weights
                if t + 1 < ntile:
                    emit_gather(t + 1)
                emit_compute(t)

            # ---- final reduction: sum over (chain, k) partitions -----
            # single partition row: [mean(nloc) | var(nloc)] (engine APs
            # must start on quadrant-aligned partitions, so no row 1)
            outs = persist.tile([1, 2 * nloc], F32, tag="outs")
            for ci in range(nred):
                o0 = ci * rhalf
                cc, off = divmod(o0, chunk)
                pr = psC.tile([1, 2, rhalf], F32, tag="C")
                for g in range(G):
                    nc.tensor.matmul(
                        pr[:, 0, :], redt[g][:, 0:1],
                        states[g][:, cc, 0, off:off + rhalf],
                        start=(g == 0), stop=(g == G - 1))
                for g in range(G):
                    nc.tensor.matmul(
                        pr[:, 1, :], redt[g][:, 1:2],
                        states[g][:, cc, 1, off:off + rhalf],
                        start=(g == 0), stop=(g == G - 1))
                nc.scalar.copy(outs[0:1, o0:o0 + rhalf], pr[:, 0, :])
                nc.scalar.copy(
                    outs[0:1, nloc + o0:nloc + o0 + rhalf], pr[:, 1, :])
            nc.sync.dma_start(out_d.rearrange("a b -> (a b)")[None, :], outs[:])

    nc.compile()
    return nc


def kernel(Xnew, meanw0, meanw, varw0, varw, prior_sc, post_prec, perm):
    uv_shards, shared, qbar = _host_tensors(
        Xnew, meanw0, meanw, varw0, varw, prior_sc, post_prec, perm)
    nc = _build_module(NLOC)
    in_maps = [dict(uv=uv_shards[i], **shared) for i in range(NCORES)]
    res = bass_utils.run_bass_kernel_spmd(
        nc, in_maps, core_ids=list(range(NCORES)))
    outs = [res.results[i]["out"] for i in range(NCORES)]
    f_mean = np.concatenate([o[0] for o in outs]).reshape(N, 1).astype(np.float32)
    f_var = (np.concatenate([o[1] for o in outs]).reshape(N, 1)
             * np.float32(qbar)).astype(np.float32)
    return f_mean, f_var
